# revision 1
# baseline (speedup 1.0000x reference)
"""PointNet Feature Propagation kernel for Trainium2 (8 NeuronCores, SPMD).

Pipeline per core (data-parallel over N: each core owns 2048 of 16384 points):
  1. Distance matrix via a single K=21 bf16 matmul per PSUM bank: coordinates
     are split hi/mid/lo (3x bf16 = 24 mantissa bits) and all significant
     cross terms are stacked along K, so psum = 2<x1,x2> - |x2|^2 to ~1e-7 --
     full fp32-grade ranking at bf16 matmul speed (PE cost is free-dim only).
  2. nc.vector.max (top-8 per row) + max_index give the 3 nearest refs per
     point-tile; indirect-DMA gathers of neighbor features are issued
     immediately so they overlap the DVE-bound selection phase.
  3. Inverse-distance weights computed batched on tiny [128,16,3] tiles.
  4. Gathered features (bf16) are weighted (per-partition tensor_scalar) and
     transposed to feature-major via PE identity matmuls with PSUM k-summing.
  5. Two 1x1-conv layers as bf16 PE matmuls; BN(inference)+ReLU fused into
     one ACT op per output chunk via per-partition scale/bias.
"""
import ml_dtypes
import numpy as np

import concourse.bacc as bacc
import concourse.bass as bass
import concourse.mybir as mybir
from concourse import bass_utils
from concourse.masks import make_identity
from concourse.tile import TileContext

f32 = mybir.dt.float32
bf16 = mybir.dt.bfloat16
u32 = mybir.dt.uint32

NCORES = 8
N = 16384
NLOC = N // NCORES          # 2048 points per core
S = 2048                    # reference points (replicated)
D1 = 128                    # points1 channels
D2 = 256                    # points2 channels
M0 = 256                    # mlp hidden
M1 = 128                    # mlp out
NT = NLOC // 128            # 16 point-tiles per core
BN_EPS = 1e-5

_CACHE = {}


def build():
    nc = bacc.Bacc("TRN2", target_bir_lowering=False)

    xz1c = nc.dram_tensor("xz1c", [16, 3 * 128], f32, kind="ExternalInput")
    xz2c = nc.dram_tensor("xz2c", [16, 3 * 128], f32, kind="ExternalInput")
    neg1h = nc.dram_tensor("neg1h", [3, NLOC], bf16, kind="ExternalInput")
    p1h = nc.dram_tensor("p1h", [D1, NLOC], f32, kind="ExternalInput")
    p2T = nc.dram_tensor("p2T", [S, D2], f32, kind="ExternalInput")
    w0h = nc.dram_tensor("w0h", [128, 3, M0], f32, kind="ExternalInput")
    w1h = nc.dram_tensor("w1h", [128, 2, M1], f32, kind="ExternalInput")
    bn0h = nc.dram_tensor("bn0h", [128, 10], f32, kind="ExternalInput")
    bn1h = nc.dram_tensor("bn1h", [128, 5], f32, kind="ExternalInput")
    out = nc.dram_tensor("out", [M1, NLOC], f32, kind="ExternalOutput")

    AL = mybir.AluOpType
    AX = mybir.AxisListType
    ACT = mybir.ActivationFunctionType

    with TileContext(nc) as tc:
        with tc.tile_pool(name="const", bufs=1) as cp:
            p1_bf = cp.tile([D1, NLOC], bf16)
            w0_sb = cp.tile([128, 3, M0], f32)
            w0_bf = cp.tile([128, 3, M0], bf16)
            w1_sb = cp.tile([128, 2, M1], f32)
            w1_bf = cp.tile([128, 2, M1], bf16)
            bn0 = cp.tile([128, 10], f32)
            bn1 = cp.tile([128, 5], f32)
            x1c = cp.tile([16, 3, 128], f32)
            x2c = cp.tile([16, 3, 128], f32)
            ident = cp.tile([128, 128], f32)
            ident_bf = cp.tile([128, 128], bf16)
            aug1 = cp.tile([24, NLOC], bf16)
            aug2 = cp.tile([24, S], bf16)

            # xyz loads first: they gate the split/repack chain -> dist matmuls
            nc.sync.dma_start(x1c[:], xz1c[:])
            nc.sync.dma_start(x2c[:], xz2c[:])
            nc.sync.dma_start(aug1[18:21, :], neg1h[:])
            # bulk loads on the Scalar HWDGE queue so they don't block repacks
            nc.scalar.dma_start(w0_sb[:], w0h[:])
            nc.scalar.dma_start(w1_sb[:], w1h[:])
            nc.scalar.dma_start(bn0[:], bn0h[:])
            nc.scalar.dma_start(bn1[:], bn1h[:])
            make_identity(nc, ident[:])
            nc.vector.tensor_copy(ident_bf[:], ident[:])
            nc.vector.tensor_copy(w0_bf[:], w0_sb[:])
            nc.vector.tensor_copy(w1_bf[:], w1_sb[:])

            with tc.tile_pool(name="prep", bufs=1) as pp:
                _q = [nc.sync, nc.scalar, nc.gpsimd]
                _qi = [0]

                def rq():
                    _qi[0] += 1
                    return _q[_qi[0] % 3]

                # hi/mid/lo bf16 splits of x1, 2*x2, |x2|^2 (packed layouts);
                # each split product is repacked to its aug K-rows right away
                # (per-coord-row DMAs, spread across the three DMA queues).
                def split3(src_ap, shape, dsts):
                    """dsts: list of 3 lists of (aug_tile, row) targets for h/m/l."""
                    h_bf = pp.tile(shape, bf16)
                    m_bf = pp.tile(shape, bf16)
                    l_bf = pp.tile(shape, bf16)
                    tf = pp.tile(shape, f32)
                    r1 = pp.tile(shape, f32)

                    def repack(bf_tile, targets):
                        for (augt, row) in targets:
                            for c in range(shape[1] if len(shape) == 3 else 1):
                                src = bf_tile[:, c, :] if len(shape) == 3 else bf_tile[:]
                                rq().dma_start(augt[row + c:row + c + 1, :], src)

                    nc.vector.tensor_copy(h_bf[:], src_ap)
                    repack(h_bf, dsts[0])
                    nc.vector.tensor_copy(tf[:], h_bf[:])
                    nc.vector.tensor_tensor(r1[:], src_ap, tf[:], op=AL.subtract)
                    nc.vector.tensor_copy(m_bf[:], r1[:])
                    repack(m_bf, dsts[1])
                    nc.vector.tensor_copy(tf[:], m_bf[:])
                    nc.vector.tensor_tensor(r1[:], r1[:], tf[:], op=AL.subtract)
                    nc.vector.tensor_copy(l_bf[:], r1[:])
                    repack(l_bf, dsts[2])

                # K-row layout: combos (lhs, rhs) of the split products:
                # rows 0-2 (h,h) 3-5 (h,m) 6-8 (m,h) 9-11 (m,m) 12-14 (h,l)
                # 15-17 (l,h) 18-20 (-1, s2 h/m/l)
                t2 = pp.tile([16, 3, 128], f32)
                nc.vector.tensor_scalar_mul(t2[:], x2c[:], 2.0)
                split3(x1c[:], [16, 3, 128],
                       [[(aug1, 0), (aug1, 3), (aug1, 12)],
                        [(aug1, 6), (aug1, 9)],
                        [(aug1, 15)]])
                split3(t2[:], [16, 3, 128],
                       [[(aug2, 0), (aug2, 6), (aug2, 15)],
                        [(aug2, 3), (aug2, 9)],
                        [(aug2, 12)]])
                q2 = pp.tile([16, 3, 128], f32)
                s2 = pp.tile([16, 128], f32)
                nc.vector.tensor_tensor(q2[:], x2c[:], x2c[:], op=AL.mult)
                nc.vector.tensor_tensor(s2[:], q2[:, 0, :], q2[:, 1, :], op=AL.add)
                nc.vector.tensor_tensor(s2[:], s2[:], q2[:, 2, :], op=AL.add)
                split3(s2[:], [16, 128],
                       [[(aug2, 18)], [(aug2, 19)], [(aug2, 20)]])

                # |x1|^2 per point, point-major [128, 16]
                q1 = pp.tile([16, 3, 128], f32)
                s1 = pp.tile([16, 128], f32)
                nc.vector.tensor_tensor(q1[:], x1c[:], x1c[:], op=AL.mult)
                nc.vector.tensor_tensor(s1[:], q1[:, 0, :], q1[:, 1, :], op=AL.add)
                nc.vector.tensor_tensor(s1[:], s1[:], q1[:, 2, :], op=AL.add)
                sq1 = cp.tile([128, 16], f32)
                with tc.tile_pool(name="pps", bufs=1, space="PSUM") as pps:
                    psq = pps.tile([128, 16], f32)
                    nc.tensor.matmul(psq[:], lhsT=s1[:], rhs=ident[0:16, 0:16],
                                     start=True, stop=True)
                    nc.scalar.copy(sq1[:], psq[:])

                # non-critical-path loads and casts after the split chain
                p1_stg = pp.tile([D1, NLOC], f32)
                nc.scalar.dma_start(p1_stg[:], p1h[:])
                nc.vector.tensor_copy(p1_bf[:], p1_stg[:])

                # bf16 copy of the gather table (points2^T)
                p2bf = nc.dram_tensor("p2bf_scratch", [S, D2], bf16, kind="Internal")
                stg = pp.tile([128, S // 128, D2], f32)
                stgb = pp.tile([128, S // 128, D2], bf16)
                nc.scalar.dma_start(stg[:], p2T[:].rearrange("(a p) c -> p a c", p=128))
                nc.vector.tensor_copy(stgb[:], stg[:])
                nc.scalar.dma_start(p2bf[:].rearrange("(a p) c -> p a c", p=128), stgb[:])

            # BN folding: y = relu(x*gam + dl)
            gam0 = cp.tile([128, 2], f32)
            dl0 = cp.tile([128, 2], f32)
            gam1 = cp.tile([128, 1], f32)
            dl1 = cp.tile([128, 1], f32)
            t0 = cp.tile([128, 2], f32)
            t1 = cp.tile([128, 1], f32)
            nc.vector.tensor_scalar_add(t0[:], bn0[:, 0:2], BN_EPS)
            nc.scalar.sqrt(t0[:], t0[:])
            nc.vector.reciprocal(t0[:], t0[:])
            nc.vector.tensor_tensor(gam0[:], t0[:], bn0[:, 2:4], op=AL.mult)
            nc.vector.tensor_tensor(dl0[:], bn0[:, 4:6], bn0[:, 6:8], op=AL.subtract)
            nc.vector.tensor_tensor(dl0[:], dl0[:], gam0[:], op=AL.mult)
            nc.vector.tensor_tensor(dl0[:], dl0[:], bn0[:, 8:10], op=AL.add)
            nc.vector.tensor_scalar_add(t1[:], bn1[:, 0:1], BN_EPS)
            nc.scalar.sqrt(t1[:], t1[:])
            nc.vector.reciprocal(t1[:], t1[:])
            nc.vector.tensor_tensor(gam1[:], t1[:], bn1[:, 1:2], op=AL.mult)
            nc.vector.tensor_tensor(dl1[:], bn1[:, 2:3], bn1[:, 3:4], op=AL.subtract)
            nc.vector.tensor_tensor(dl1[:], dl1[:], gam1[:], op=AL.mult)
            nc.vector.tensor_tensor(dl1[:], dl1[:], bn1[:, 4:5], op=AL.add)

            mx = cp.tile([128, NT, 8], f32)
            ix = cp.tile([128, NT, 8], u32)
            g_all = cp.tile([128, NT, 3, D2], bf16)

            # ---- KNN: distances + top-3 + gathers (gathers overlap DVE) ----
            with tc.tile_pool(name="kps", bufs=2, space="PSUM") as kps:
                for t in range(NT):
                    pk = kps.tile([128, S], f32, tag="pk")
                    for b in range(4):
                        nc.tensor.matmul(
                            pk[:, 512 * b:512 * (b + 1)],
                            lhsT=aug1[0:21, 128 * t:128 * (t + 1)],
                            rhs=aug2[0:21, 512 * b:512 * (b + 1)],
                            start=True, stop=True)
                    nc.vector.max(out=mx[:, t, :], in_=pk[:])
                    nc.vector.max_index(out=ix[:, t, :], in_max=mx[:, t, :], in_values=pk[:])
                    for k in range(3):
                        nc.gpsimd.indirect_dma_start(
                            out=g_all[:, t, k, :], out_offset=None, in_=p2bf[:],
                            in_offset=bass.IndirectOffsetOnAxis(
                                ap=ix[:, t, k:k + 1], axis=0))

            # ---- weights (batched small ops) ----
            # d3 = |x1|^2 - psumtop3 ; w = (1/(d3+1e-8)) normalized
            d3 = cp.tile([128, NT, 3], f32)
            w3 = cp.tile([128, NT, 3], f32)
            ws = cp.tile([128, NT], f32)
            nc.vector.tensor_tensor(d3[:], sq1[:].to_broadcast([128, NT, 3]),
                                    mx[:, :, 0:3], op=AL.subtract)
            nc.vector.tensor_scalar_add(d3[:], d3[:], 1e-8)
            nc.vector.reciprocal(w3[:], d3[:])
            nc.vector.reduce_sum(ws[:], w3[:], axis=AX.X)
            nc.vector.reciprocal(ws[:], ws[:])
            nc.vector.tensor_tensor(w3[:], w3[:], ws[:].to_broadcast([128, NT, 3]),
                                    op=AL.mult)

            # ---- weight + transpose to feature-major ----
            # out = g_chunk.T @ diag(w): transpose and per-point weighting in
            # one PE matmul, k-summed in PSUM.
            interp = cp.tile([128, 2, NLOC], bf16)
            with tc.tile_pool(name="ips", bufs=2, space="PSUM") as ips, \
                 tc.tile_pool(name="dgp", bufs=6) as dgp:
                for t in range(NT):
                    dgs = []
                    for k in range(3):
                        dg = dgp.tile([128, 128], bf16, tag=f"dg{k}")
                        nc.vector.tensor_scalar_mul(
                            dg[:], ident_bf[:], w3[:, t, k:k + 1])
                        dgs.append(dg)
                    psI = ips.tile([128, 2, 128], f32, tag="psI")
                    for h in range(2):
                        for k in range(3):
                            nc.tensor.matmul(
                                psI[:, h, :],
                                lhsT=g_all[:, t, k, 128 * h:128 * (h + 1)],
                                rhs=dgs[k][:],
                                start=(k == 0), stop=(k == 2))
                    nc.scalar.copy(interp[:, :, 128 * t:128 * (t + 1)], psI[:])

            # ---- MLP (bf16 operands, fp32 accumulate) ----
            h0 = cp.tile([128, 2, NLOC], bf16)
            out_sb = cp.tile([128, NLOC], f32)
            with tc.tile_pool(name="mps", bufs=1, space="PSUM") as mps:
                for m in range(2):
                    ps0 = mps.tile([128, NLOC], f32, tag="ps0")
                    for ki in range(3):
                        rhs = p1_bf[:] if ki == 0 else interp[:, ki - 1, :]
                        for b in range(4):
                            nc.tensor.matmul(
                                ps0[:, 512 * b:512 * (b + 1)],
                                lhsT=w0_bf[:, ki, 128 * m:128 * (m + 1)],
                                rhs=rhs[:, 512 * b:512 * (b + 1)],
                                start=(ki == 0), stop=(ki == 2))
                    nc.scalar.activation(out=h0[:, m, :], in_=ps0[:], func=ACT.Relu,
                                         bias=dl0[:, m:m + 1], scale=gam0[:, m:m + 1])
                ps1 = mps.tile([128, NLOC], f32, tag="ps1")
                for ki in range(2):
                    for b in range(4):
                        nc.tensor.matmul(
                            ps1[:, 512 * b:512 * (b + 1)],
                            lhsT=w1_bf[:, ki, :],
                            rhs=h0[:, ki, 512 * b:512 * (b + 1)],
                            start=(ki == 0), stop=(ki == 1))
                nc.scalar.activation(out=out_sb[:], in_=ps1[:], func=ACT.Relu,
                                     bias=dl1[:, 0:1], scale=gam1[:, 0:1])
            nc.sync.dma_start(out[:], out_sb[:])

    nc.finalize()
    return nc


def make_in_maps(inputs):
    xyz1 = np.asarray(inputs["xyz1"], np.float32)
    xyz2 = np.asarray(inputs["xyz2"], np.float32)
    points1 = np.asarray(inputs["points1"], np.float32)
    points2 = np.asarray(inputs["points2"], np.float32)
    W0 = np.asarray(inputs["W0"], np.float32)
    W1 = np.asarray(inputs["W1"], np.float32)

    p2T = np.ascontiguousarray(points2.T)
    w0h = np.ascontiguousarray(W0.reshape(3, 128, M0).transpose(1, 0, 2))
    w1h = np.ascontiguousarray(W1.reshape(2, 128, M1).transpose(1, 0, 2))
    xz2c = np.ascontiguousarray(
        xyz2.reshape(3, 16, 128).transpose(1, 0, 2).reshape(16, -1))
    neg1h = np.full((3, NLOC), -1.0, dtype=ml_dtypes.bfloat16)

    def col(v, parts):
        return np.ascontiguousarray(np.asarray(v, np.float32).reshape(parts, 128).T)

    bn0h = np.concatenate([col(inputs[k], 2) for k in ("v0", "g0", "b0", "m0", "be0")], axis=1)
    bn1h = np.concatenate([col(inputs[k], 1) for k in ("v1", "g1", "b1", "m1", "be1")], axis=1)

    in_maps = []
    for c in range(NCORES):
        sl = slice(c * NLOC, (c + 1) * NLOC)
        x1 = xyz1[:, sl]
        xz1c = np.ascontiguousarray(
            x1.reshape(3, 16, NLOC // 16).transpose(1, 0, 2).reshape(16, -1))
        in_maps.append(dict(
            xz1c=xz1c, xz2c=xz2c, neg1h=neg1h,
            p1h=np.ascontiguousarray(points1[:, sl]),
            p2T=p2T, w0h=w0h, w1h=w1h, bn0h=bn0h, bn1h=bn1h,
        ))
    return in_maps


def run(inputs, trace=False, **kwargs):
    if "nc" not in _CACHE:
        _CACHE["nc"] = build()
    nc = _CACHE["nc"]
    in_maps = make_in_maps(inputs)
    res = bass_utils.run_bass_kernel_spmd(
        nc, in_maps, core_ids=list(range(NCORES)), trace=trace, **kwargs)
    outs = [res.results[c]["out"] for c in range(NCORES)]
    full = np.concatenate(outs, axis=1)
    return full, res


def kernel(**inputs):
    full, _ = run(inputs, trace=False)
    return full

